# revision 1
# baseline (speedup 1.0000x reference)
"""HMM forward (CgpHmmCell) Trainium2 kernel, v2.

Strategy (vs v1): host supplies x pre-transposed to [m, t, b] fp8 so the
emission E^T = Bm^T @ x_t is ONE matmul per 2 steps with m on partitions --
no PE transposes, no f32->bf16 conversion pass, 4x less DMA.

  - 8 cores x 2 time-chains per core; chain (k, c) covers global t-span
    (512k + 256c, 512k + 256(c+1)], warm-started W=16 steps earlier
    (chain (0,0) starts exactly at t=0 from the true I).
  - State vT[s + 64h, j] (s on partitions, b = 256h + j), bf16.
  - Per step: u = (64 A)^T v via one block-diag matmul [128,128]x[128,256];
    v = u (PSUM f32) * ets (SBUF bf16) in one DVE tensor_tensor.
  - Emissions: et2[s+64h, dt, j] = sum_m Bm[m, s] x[m, t+dt, 256h+j], two
    N=512 matmuls per 2 steps; ScalarE copies PSUM->SBUF bf16.
  - Every RESC=16 steps the per-sequence mass Z[b] is probed (ones-matmuls,
    b lands on partitions), stored as bf16 reciprocal slots (the output),
    broadcast back to [s, b] layout via PE transpose + rank-1 matmuls, and
    multiplied into v two steps later.  Host sums -log(slots) per chain.

Self-contained: hardcodes shapes for the 512x4096x125/S=64 problem.
"""

import numpy as np

import concourse.bass as bass
import concourse.tile as tile
from concourse import bacc, mybir
from concourse import bass_utils

B, T, S, M = 512, 4096, 64, 125
NCORES = 8
NCHAIN = 2
SPAN = T // (NCORES * NCHAIN)   # 256
WARM = 16
NSTEP = SPAN + WARM             # 272 recurrence steps per chain
NT = NSTEP + 1                  # 273 t-positions per chain
CHUNK_T = 16                    # t per DMA chunk
C_PRE = 64.0                    # transition prescale (power of two, exact)
RESC = 16                       # rescale/probe period
LAG = 2                         # probe applied to v LAG steps later
NSLOT = NSTEP // RESC + 1       # 17 periodic probes + 1 extra at NSTEP-1

F32 = mybir.dt.float32
BF16 = mybir.dt.bfloat16
FP8 = mybir.dt.float8e4


def _build_program(reps=1):
    nc = bacc.Bacc("TRN2", target_bir_lowering=False, debug=False,
                   num_devices=NCORES)

    x_d = nc.dram_tensor("x", [M, NCHAIN, NT, B], FP8, kind="ExternalInput")
    icol_d = nc.dram_tensor("icol", [128, NCHAIN], F32, kind="ExternalInput")
    apre_d = nc.dram_tensor("apre", [128, 128], BF16, kind="ExternalInput")
    bm_d = nc.dram_tensor("bm", [M, S], BF16, kind="ExternalInput")
    ident_d = nc.dram_tensor("ident", [128, 128], F32, kind="ExternalInput")
    ones_d = nc.dram_tensor("ones", [128, 128], BF16, kind="ExternalInput")
    ones32_d = nc.dram_tensor("ones32", [1, 64], F32, kind="ExternalInput")
    out_d = nc.dram_tensor("slots", [128, NCHAIN * 4 * NSLOT], F32,
                           kind="ExternalOutput")

    with tile.TileContext(nc) as tc:
        with (
            tc.tile_pool(name="const", bufs=1) as constp,
            tc.tile_pool(name="xstage", bufs=3) as xstagep,
            tc.tile_pool(name="ets", bufs=2) as etsp,
            tc.tile_pool(name="state", bufs=1) as statep,
            tc.tile_pool(name="etp", bufs=2, space="PSUM") as etpp,
            tc.tile_pool(name="up", bufs=1, space="PSUM") as upp,
            tc.tile_pool(name="zp", bufs=1, space="PSUM") as zpp,
        ):
            icol = constp.tile([128, NCHAIN], F32)
            apre = constp.tile([128, 128], BF16)
            bm = constp.tile([M, S], BF16)
            ident = constp.tile([128, 128], F32)
            ones = constp.tile([128, 128], BF16)
            ones32 = constp.tile([1, 64], F32)
            nc.sync.dma_start(icol[:], icol_d.ap())
            nc.sync.dma_start(apre[:], apre_d.ap())
            nc.sync.dma_start(bm[:], bm_d.ap())
            nc.sync.dma_start(ident[:], ident_d.ap())
            nc.sync.dma_start(ones[:], ones_d.ap())
            nc.sync.dma_start(ones32[:], ones32_d.ap())

            # per-chain persistent state
            v = [statep.tile([128, 256], BF16, name=f"v{c}")
                 for c in range(NCHAIN)]
            slots = statep.tile([128, NCHAIN, 4, NSLOT], F32)

            import contextlib
            loop_cm = (tc.For_i(0, reps, 1) if reps > 1
                       else contextlib.nullcontext())
            with loop_cm:
                _emit_body(nc, tc, locals())

            nc.sync.dma_start(out_d.ap(),
                              slots[:].rearrange("p c g r -> p (c g r)"))

    nc.compile()
    return nc


def _emit_body(nc, tc, env):
    icol, apre, bm, ident, ones, ones32 = (
        env["icol"], env["apre"], env["bm"], env["ident"], env["ones"],
        env["ones32"])
    v, slots, x_d = env["v"], env["slots"], env["x_d"]
    xstagep, etsp = env["xstagep"], env["etsp"]
    etpp, upp, zpp = env["etpp"], env["upp"], env["zpp"]

    x_ch = [None] * NCHAIN     # current staged x chunk per chain
    ets_ch = [None] * NCHAIN   # current emission pair per chain
    bc_ch = [None] * NCHAIN    # pending rescale broadcast per chain

    for j in range(NT):
        ct0, tt = divmod(j, CHUNK_T)
        for c in range(NCHAIN):
            if tt == 0:
                ct = min(CHUNK_T, NT - ct0 * CHUNK_T)
                t0c = ct0 * CHUNK_T
                x_st = xstagep.tile([M, CHUNK_T, B], FP8, tag=f"x{c}")
                nc.sync.dma_start(x_st[:, :ct, :],
                                  x_d.ap()[:, c, t0c:t0c + ct, :])
                x_ch[c] = x_st

            if j % 2 == 0:
                ndt = 2 if j + 1 < NT else 1
                et2 = etpp.tile([128, 2, 256], F32, tag=f"et{c}")
                for h in range(2):
                    nc.tensor.matmul(
                        out=et2[64 * h:64 * h + 64, :ndt, :],
                        lhsT=bm[:],
                        rhs=x_ch[c][:, tt:tt + ndt, 256 * h:256 * h + 256])
                ets = etsp.tile([128, 2, 256], BF16, tag=f"ets{c}")
                # PSUM->SBUF; on rescale steps fold the pending per-b
                # mass correction into the emission instead of a separate
                # multiply on v.
                if bc_ch[c] is not None:
                    nc.vector.tensor_mul(ets[:, 0, :], et2[:, 0, :],
                                         bc_ch[c][:])
                    if ndt > 1:
                        nc.vector.tensor_copy(ets[:, 1, :], et2[:, 1, :])
                    bc_ch[c] = None
                else:
                    nc.vector.tensor_copy(ets[:, :ndt, :], et2[:, :ndt, :])
                ets_ch[c] = ets

            par = j % 2
            if j == 0:
                # v = E_0^T * init (per-partition scalar broadcast over b)
                nc.vector.tensor_scalar_mul(v[c][:], ets_ch[c][:, 0, :],
                                            icol[:, c:c + 1])
            else:
                u = upp.tile([128, 256], F32, tag=f"u{c}")
                nc.tensor.matmul(out=u[:], lhsT=apre[:], rhs=v[c][:])
                # v = u * E^T  (PSUM f32 x SBUF bf16 -> SBUF bf16)
                nc.vector.tensor_mul(v[c][:], u[:], ets_ch[c][:, par, :])

                probe = (j % RESC == 0) or (j == NSTEP - 1)
                if probe:
                    r = (j // RESC - 1) if j % RESC == 0 else NSLOT - 1
                    # one PSUM bank per chain holds zp, rt and bc scratch;
                    # their liveness within a probe is strictly sequential.
                    pz = zpp.tile([128, 512], F32, tag=f"z{c}")
                    zp = pz[:, 0:4]
                    for g in range(4):
                        hb = 64 * (g // 2)
                        nc.tensor.matmul(
                            out=zp[:, g:g + 1],
                            lhsT=v[c][hb:hb + 64,
                                      128 * (g % 2):128 * (g % 2) + 128],
                            rhs=ones[hb:hb + 64, 0:1])
                    nc.vector.reciprocal(slots[:, c, :, r], zp)
                    if j % RESC == 0 and j <= NSTEP - RESC:
                        # rebroadcast recip to [s, b] layout for the apply:
                        # 4 col-transposes land recip on partition 0 as
                        # [1, 512], then 4 rank-1 matmuls broadcast each
                        # 128-col strip to its 64-partition block.
                        for g in range(4):
                            nc.tensor.transpose(
                                out=pz[0:1, 128 * g:128 * g + 128],
                                in_=slots[:, c, g:g + 1, r],
                                identity=ident[:])
                        rts = etsp.tile([1, 512], F32, tag=f"rts{c}")
                        nc.vector.tensor_copy(rts[:], pz[0:1, :])
                        bcp = pz[:, 0:256]
                        for g in range(4):
                            nc.tensor.matmul(
                                out=bcp[64 * (g // 2):64 * (g // 2) + 64,
                                        128 * (g % 2):128 * (g % 2) + 128],
                                lhsT=ones32[0:1, :],
                                rhs=rts[0:1, 128 * g:128 * g + 128])
                        bcs = etsp.tile([128, 256], BF16, tag=f"bcs{c}")
                        nc.vector.tensor_copy(bcs[:], bcp[:])
                        bc_ch[c] = bcs


_NC_CACHE = None


def _get_program():
    global _NC_CACHE
    if _NC_CACHE is None:
        _NC_CACHE = _build_program()
    return _NC_CACHE


def _to_bf16(a):
    import ml_dtypes
    return np.asarray(a, np.float32).astype(ml_dtypes.bfloat16)


def _host_inputs(x, I, A, Bm):
    """Per-core in_maps. x transposed to [m, chain, t, b] fp8; constants
    replicated."""
    import ml_dtypes
    x = np.asarray(x, np.float32)
    I = np.asarray(I, np.float32).reshape(1, S)
    A = np.asarray(A, np.float32)
    Bm = np.asarray(Bm, np.float32)

    # [M, T, B] fp8 once; per-chain slices after
    xT8 = np.ascontiguousarray(x.transpose(2, 1, 0)).astype(
        ml_dtypes.float8_e4m3)

    bd = np.zeros((128, 128), np.float32)      # block-diag: one matmul
    bd[:S, :S] = A * C_PRE
    bd[S:, S:] = A * C_PRE
    apre = _to_bf16(bd)
    bm_b = _to_bf16(Bm)                        # [m, s]
    ident = np.eye(128, dtype=np.float32)
    ones = _to_bf16(np.ones((128, 128), np.float32))
    ones32 = np.ones((1, 64), np.float32)
    icol_real = np.concatenate([I.T, I.T], axis=0).astype(np.float32)

    in_maps = []
    for k in range(NCORES):
        xs = np.empty((M, NCHAIN, NT, B), ml_dtypes.float8_e4m3)
        icol = np.ones((128, NCHAIN), np.float32)
        for c in range(NCHAIN):
            t0 = 0 if (k == 0 and c == 0) else SPAN * (2 * k + c) - WARM
            ts = np.clip(np.arange(t0, t0 + NT), 0, T - 1)
            xs[:, c] = xT8[:, ts, :]
            if k == 0 and c == 0:
                icol[:, 0] = icol_real[:, 0]
        in_maps.append({
            "x": xs,
            "icol": icol,
            "apre": apre,
            "bm": bm_b,
            "ident": ident,
            "ones": ones,
            "ones32": ones32,
        })
    return in_maps


def _host_reduce(results):
    """Combine per-core slot outputs into ll [B, 1] float32."""
    lnc = np.log(np.float64(C_PRE))
    total = np.zeros((B, 1), np.float64)
    for k in range(NCORES):
        sl = np.asarray(results[k]["slots"], np.float32).reshape(
            128, NCHAIN, 4, NSLOT).astype(np.float64)
        logm = -np.log(sl)                     # [128 p, chain, 4 g, NSLOT]
        for c in range(NCHAIN):
            if k == 0 and c == 0:
                contrib = logm[:, c, :, 0:16].sum(axis=2) - SPAN * lnc
            elif not (k == NCORES - 1 and c == NCHAIN - 1):
                contrib = logm[:, c, :, 1:17].sum(axis=2) - SPAN * lnc
            else:
                contrib = (logm[:, c, :, 1:16].sum(axis=2)
                           + logm[:, c, :, NSLOT - 1]) - (SPAN - 1) * lnc
            # b = 128*g + p
            total += contrib.T.reshape(B, 1)
    return total.astype(np.float32)


def kernel(x, I, A, Bm):
    nc = _get_program()
    in_maps = _host_inputs(x, I, A, Bm)
    res = bass_utils.run_bass_kernel_spmd(nc, in_maps,
                                          core_ids=list(range(NCORES)))
    return _host_reduce(res.results)



# revision 19
# speedup vs baseline: 4.1370x; 4.1370x over previous
"""HMM forward (CgpHmmCell) Trainium2 kernel, v3.

Architecture (vs v2): 8 cores x 4 time-chains per core, span 128, warm 8.
NO on-device rescaling at all: per-chain host-calibrated scale s_c is folded
into the fp8 emission matrix so the per-step mass drift is centered at 2^0;
the mass random-walks +-~15 bits over a chain, well within bf16/f32 range.
Probes just READ cumulative mass M_j (4 ones-matmuls) at 4 fixed positions;
host reconstructs ll = log M_hi - log M_lo - nsteps*log s_c.

Per position j (t advances by 1) per chain: one fp8 emission matmul pair
(Bm^T x one-hot, contraction 126 on partitions), one bf16 block-diag
transition matmul [128,128]x[128,256], one elementwise v = u(PSUM) *
ets(SBUF bf16). The
HW allows only ONE PSUM operand per vector op, so the emission PSUM tile
is drained to SBUF by a single ScalarE (ACT) copy [128,1024] per position
(ACT is otherwise idle). Chains grouped {0,1} | {2,3}; each group's
multiply is column-split across DVE and Pool (GPSIMD) so both vector
engines run every position while the two groups' recurrences pipeline
against PE.

Self-contained: hardcodes shapes for the 512x4096x125/S=64 problem.
"""

import numpy as np

import concourse.bass as bass
import concourse.tile as tile
from concourse import bacc, mybir
from concourse import bass_utils

B, T, S, M = 512, 4096, 64, 125
NCORES = 8
NCH = 4                      # chains per core
NCHAIN = NCORES * NCH        # 32
SPAN = T // NCHAIN           # 128
WARM = 4
NSTEP = SPAN + WARM          # 132 recurrence steps per chain
NT = NSTEP + 1               # 137 t-positions per chain
CT = 8                       # t per DMA chunk
PROBE_J = [WARM, SPAN, NSTEP - 1, NSTEP]   # j = 8, 128, 135, 136
NPROBE = len(PROBE_J)
SPL = 320                    # DVE columns of each group's 512-wide multiply
GROUPS = [(0, 2), (2, 4)]    # chain ranges per pipeline group

F32 = mybir.dt.float32
BF16 = mybir.dt.bfloat16
FP8 = mybir.dt.float8e4


def _build_program(reps=1):
    nc = bacc.Bacc("TRN2", target_bir_lowering=False, debug=False,
                   num_devices=NCORES)

    x_d = nc.dram_tensor("x", [126, NCH, NT, B], FP8, kind="ExternalInput")
    bm2_d = nc.dram_tensor("bm2", [126, NCH, S], FP8, kind="ExternalInput")
    apre_d = nc.dram_tensor("apre", [128, 128], BF16, kind="ExternalInput")
    icol_d = nc.dram_tensor("icol", [128, NCH], F32, kind="ExternalInput")
    ones_d = nc.dram_tensor("ones", [128, 1], BF16, kind="ExternalInput")
    out_d = nc.dram_tensor("slots", [128, NCH * NPROBE * 4], F32,
                           kind="ExternalOutput")

    with tile.TileContext(nc) as tc:
        with (
            tc.tile_pool(name="const", bufs=1) as constp,
            tc.tile_pool(name="xstage", bufs=3) as xstagep,
            tc.tile_pool(name="state", bufs=1) as statep,
            tc.tile_pool(name="etsb", bufs=2) as etsbp,
            tc.tile_pool(name="etp", bufs=2, space="PSUM") as etp,
            tc.tile_pool(name="up", bufs=1, space="PSUM") as up,
            tc.tile_pool(name="zp", bufs=1, space="PSUM") as zp,
        ):
            bm2 = constp.tile([126, NCH, S], FP8)
            apre = constp.tile([128, 128], BF16)
            icol = constp.tile([128, NCH], F32)
            ones = constp.tile([128, 1], BF16)
            nc.sync.dma_start(bm2[:], bm2_d.ap())
            nc.sync.dma_start(apre[:], apre_d.ap())
            nc.sync.dma_start(icol[:], icol_d.ap())
            nc.sync.dma_start(ones[:], ones_d.ap())

            v = statep.tile([128, NCH * 256], BF16)
            slots = statep.tile([128, NCH, NPROBE, 4], F32)

            import contextlib
            loop_cm = (tc.For_i(0, reps, 1) if reps > 1
                       else contextlib.nullcontext())
            with loop_cm:
                _emit_body(nc, tc, locals())

            nc.sync.dma_start(out_d.ap(),
                              slots[:].rearrange("p c q g -> p (c q g)"))

    nc.compile()
    return nc


def _emit_body(nc, tc, env):
    bm2, apre, icol, ones = env["bm2"], env["apre"], env["icol"], env["ones"]
    v, slots, x_d = env["v"], env["slots"], env["x_d"]
    xstagep, etsbp, etp, up = (env["xstagep"], env["etsbp"], env["etp"],
                               env["up"])

    x_st = [None]          # current staged x chunk (boxed for closure)
    box = {}               # j -> (et_psum, ets_sbuf)
    zp = env["zp"]

    def stage(j):
        ct0, tt = divmod(j, CT)
        if tt == 0:
            ct = min(CT, NT - ct0 * CT)
            xt = xstagep.tile([126, NCH, CT, B], FP8, tag="x")
            nc.sync.dma_start(xt[:, :, :ct, :],
                              x_d.ap()[:, :, ct0 * CT:ct0 * CT + ct, :])
            x_st[0] = xt
        return x_st[0], tt

    def emissions(j, drain=True):
        """8 fp8 matmuls -> et[128, NCH, 256] PSUM, then one ACT copy
        draining it to SBUF bf16 (the vector ops may read at most one
        PSUM operand, which the multiply spends on u)."""
        xt, tt = stage(j)
        et = etp.tile([128, NCH, 256], F32, tag="et")
        for c in range(NCH):
            for h in range(2):
                nc.tensor.matmul(
                    out=et[64 * h:64 * h + 64, c, :],
                    lhsT=bm2[:, c, :],
                    rhs=xt[:, c, tt, 256 * h:256 * h + 256])
        es = None
        if drain:
            es = etsbp.tile([128, NCH, 256], BF16, tag="ets")
            nc.scalar.copy(es[:], et[:])
        box[j] = (et, es)

    # ---- j = 0: init v = E(t0) * icol (reads the PSUM tile directly) ----
    emissions(0, drain=False)
    et0, _ = box.pop(0)
    for c in range(NCH):
        nc.vector.tensor_scalar_mul(
            v[:, 256 * c:256 * c + 256], et0[:, c, :], icol[:, c:c + 1])
    emissions(1)

    # ---- steady state ----
    for j in range(1, NSTEP + 1):
        _, es = box.pop(j)
        ef = es[:].rearrange("p c n -> p (c n)")
        # emissions for the NEXT position first, so PE never head-blocks
        if j + 1 <= NSTEP:
            emissions(j + 1)
        us = {}
        for gi, (c0, c1) in enumerate(GROUPS):
            u = up.tile([128, 2, 256], F32, tag=f"u{gi}")
            for c in range(c0, c1):
                nc.tensor.matmul(out=u[:, c - c0, :], lhsT=apre[:],
                                 rhs=v[:, 256 * c:256 * c + 256])
            us[gi] = u
        for gi, (c0, c1) in enumerate(GROUPS):
            uf = us[gi][:].rearrange("p c n -> p (c n)")
            eg = ef[:, 512 * gi:512 * gi + 512]
            vg = v[:, 512 * gi:512 * gi + 512]
            nc.vector.tensor_mul(vg[:], uf[:], eg[:])

        if j in PROBE_J:
            pi = PROBE_J.index(j)
            zt = zp.tile([128, NCH, 4], F32, tag="z")
            for c in range(NCH):
                for g in range(4):
                    hb = 64 * (g // 2)
                    nc.tensor.matmul(
                        out=zt[:, c, g:g + 1],
                        lhsT=v[hb:hb + 64,
                               256 * c + 128 * (g % 2):
                               256 * c + 128 * (g % 2) + 128],
                        rhs=ones[hb:hb + 64, :])
            nc.vector.tensor_copy(slots[:, :, pi, :], zt[:, :, :])


# ---------------- host side ----------------

_NC_CACHE = None
_CAL_CACHE = None


def _get_program():
    global _NC_CACHE
    if _NC_CACHE is None:
        _NC_CACHE = _build_program()
    return _NC_CACHE


def _chain_t0(ci):
    return 0 if ci == 0 else SPAN * ci - WARM


def _calibrate(obs, A, Bm):
    """Per-chain scale s_c = exp(-mean ln Zraw) over the chain window."""
    pi = np.full(S, 1.0 / S, np.float64)
    Ad = A.astype(np.float64)
    for _ in range(200):
        pi = pi @ Ad
    w = Bm.astype(np.float64) @ (Ad.T @ pi)          # [M]
    lnz = np.log(w)[obs]                             # [B, T]
    ln_s = np.empty(NCHAIN, np.float64)
    for ci in range(NCHAIN):
        t0 = _chain_t0(ci)
        hi = min(SPAN * (ci + 1), T)
        ln_s[ci] = -lnz[:, max(t0, 0):hi].mean()
    return ln_s


def _host_inputs(x, I, A, Bm):
    import ml_dtypes
    x = np.asarray(x, np.float32)
    I = np.asarray(I, np.float32).reshape(S)
    A = np.asarray(A, np.float32)
    Bm = np.asarray(Bm, np.float32)

    obs = np.argmax(x, axis=2)                       # [B, T] int
    ln_s = _calibrate(obs, A, Bm)

    # one-hot [126, T, B] fp8 (padded to 126 partitions)
    xT8 = np.zeros((126, T, B), ml_dtypes.float8_e4m3)
    np.put_along_axis(xT8, obs.T[None].astype(np.int64), np.float32(1.0),
                      axis=0)

    ab = A.astype(ml_dtypes.bfloat16).astype(np.float32)
    bd = np.zeros((128, 128), np.float32)
    bd[:S, :S] = ab
    bd[S:, S:] = ab
    apre = bd.astype(ml_dtypes.bfloat16)

    Bm126 = np.zeros((126, S), np.float64)
    Bm126[:M] = Bm.astype(np.float64)

    ones = np.ones((128, 1), np.float32).astype(ml_dtypes.bfloat16)

    in_maps = []
    for k in range(NCORES):
        xs = np.empty((126, NCH, NT, B), ml_dtypes.float8_e4m3)
        bm2 = np.empty((126, NCH, S), ml_dtypes.float8_e4m3)
        icol = np.ones((128, NCH), np.float32)
        for c in range(NCH):
            ci = NCH * k + c
            t0 = _chain_t0(ci)
            ts = np.clip(np.arange(t0, t0 + NT), 0, T - 1)
            xs[:, c] = xT8[:, ts, :]
            bm2[:, c] = (Bm126 * np.exp(ln_s[ci])).astype(
                ml_dtypes.float8_e4m3)
            if ci == 0:
                icol[:S, 0] = I
                icol[S:, 0] = I
        in_maps.append({
            "x": xs,
            "bm2": bm2,
            "apre": apre,
            "icol": icol,
            "ones": ones,
        })
    return in_maps, ln_s


def _host_reduce(results, ln_s):
    """ll[b] = sum over chains of log M_hi - log M_lo - nsteps*ln s."""
    total = np.zeros(B, np.float64)
    for k in range(NCORES):
        sl = np.asarray(results[k]["slots"], np.float32).reshape(
            128, NCH, NPROBE, 4).astype(np.float64)
        # b = 128*g + p
        Mv = np.log(sl).transpose(3, 0, 1, 2).reshape(B, NCH, NPROBE)
        for c in range(NCH):
            ci = NCH * k + c
            if ci == 0:
                # probe idx 1 is j=SPAN; init counts as one application
                total += Mv[:, c, 1] - (SPAN + 1) * ln_s[ci]
            elif ci == NCHAIN - 1:
                # probe idx 2 is j=NSTEP-1 (t = T-1)
                total += (Mv[:, c, 2] - Mv[:, c, 0]
                          - (SPAN - 1) * ln_s[ci])
            else:
                total += (Mv[:, c, 3] - Mv[:, c, 0]
                          - SPAN * ln_s[ci])
    return total.reshape(B, 1).astype(np.float32)


def kernel(x, I, A, Bm):
    nc = _get_program()
    in_maps, ln_s = _host_inputs(x, I, A, Bm)
    res = bass_utils.run_bass_kernel_spmd(nc, in_maps,
                                          core_ids=list(range(NCORES)))
    return _host_reduce(res.results, ln_s)
